# revision 4
# baseline (speedup 1.0000x reference)
"""Trainium2 Bass kernel for nn_ContrastiveLoss (N=4096, D=128, NT=512, Q=8).

Strategy (8 NeuronCores, data parallel over N, no cross-core collective):
  - Host sorts rows of x by track id (the loss is invariant under a common
    permutation of rows) and, per core, permutes the COLUMN order of both
    similarity matrices so that every data-dependent gather becomes a
    compile-time-static slice:
      * x-columns: each 128-row chunk's same-track partner columns are
        placed in a fixed 256-wide window; windows live in cols [0:1024).
      * y-columns: each chunk's own-track view columns likewise.
  - Device work per core is a clean 16-phase pipeline: [128,2048] matmul
    (bf16, K=D=128) -> exp on ACT -> row-sum on DVE.  Masked corrections
    (same-track x sum, own-track y sum, min over own views) are three
    fused tensor_tensor_reduce ops on static [128,512] window slices.
  - No ACT accumulator reads, no full-width mask pass, no on-device
    logs/moments: the device ships raw per-row partials [128,28] and the
    host assembles den/num and evaluates the loss with the (validated)
    log1p series in float64, with an exact numpy fallback.
"""

import numpy as np
import ml_dtypes

import concourse.bass as bass
import concourse.bacc as bacc
import concourse.tile as tile
import concourse.mybir as mybir
from concourse import bass_utils

P = 128           # partitions / rows per chunk
N = 4096          # total rows of x
D = 128           # feature dim
NT = 512          # number of tracks
Q = 8             # views per track
CORES = 8
R = N // CORES    # rows per core = 512
NCH = R // P      # chunks per core = 4
TEMP = 0.05
INV_T = 1.0 / TEMP
W = 2048          # phase width (4 PSUM banks)
WINW = 256        # per-chunk window width
REG = 512         # mask region width (two windows)
OFFS = (0, 0, 256, 512)   # mask region start per chunk
OUTC = 28         # 16 phase row-sums + 4 samex + 4 own10 + 4 pmin
F32 = mybir.dt.float32
BF16 = mybir.dt.bfloat16
AX = mybir.AxisListType
ALU = mybir.AluOpType
ACTF = mybir.ActivationFunctionType

_CACHE = {}


def _build():
    nc = bacc.Bacc("TRN2", target_bir_lowering=False, debug=False,
                   num_devices=CORES)

    xT_d = nc.dram_tensor("xT", [D, N], BF16, kind="ExternalInput")
    yT_d = nc.dram_tensor("yT", [D, N], BF16, kind="ExternalInput")
    xsh_d = nc.dram_tensor("xsh", [D, R], BF16, kind="ExternalInput")
    xm_d = nc.dram_tensor("xm", [P, NCH * REG], BF16, kind="ExternalInput")
    ym_d = nc.dram_tensor("ym", [P, NCH * REG], BF16, kind="ExternalInput")
    out_d = nc.dram_tensor("out", [P, OUTC], F32, kind="ExternalOutput")

    with tile.TileContext(nc) as tc:
        with (
            tc.tile_pool(name="persist", bufs=1) as pp,
            tc.tile_pool(name="ef32", bufs=2) as ef,
            tc.tile_pool(name="ebf16", bufs=2) as eh,
            tc.tile_pool(name="junk", bufs=2) as jp,
            tc.tile_pool(name="psum", bufs=2, space="PSUM") as psp,
        ):
            xT_s = pp.tile([D, N], BF16, tag="xT_s")
            yT_s = pp.tile([D, N], BF16, tag="yT_s")
            xsh_s = pp.tile([D, R], BF16, tag="xsh_s")
            xm_s = pp.tile([P, NCH * REG], BF16, tag="xm_s")
            ym_s = pp.tile([P, NCH * REG], BF16, tag="ym_s")
            out_s = pp.tile([P, OUTC], F32, tag="out_s")

            # input loads, in consumption order (single HWDGE queue)
            nc.sync.dma_start(out=xsh_s[:], in_=xsh_d.ap())
            nc.sync.dma_start(out=xT_s[:, 0:W], in_=xT_d.ap()[:, 0:W])
            nc.sync.dma_start(out=xT_s[:, W:N], in_=xT_d.ap()[:, W:N])
            nc.sync.dma_start(out=yT_s[:, 0:W], in_=yT_d.ap()[:, 0:W])
            nc.sync.dma_start(out=xm_s[:], in_=xm_d.ap())
            nc.sync.dma_start(out=yT_s[:, W:N], in_=yT_d.ap()[:, W:N])
            nc.sync.dma_start(out=ym_s[:], in_=ym_d.ap())

            for cc in range(NCH):
                lhsT = xsh_s[:, cc * P:(cc + 1) * P]
                off = OFFS[cc]
                for si, src in enumerate((xT_s, yT_s)):
                    for h in range(2):
                        ps = psp.tile([P, W], F32, tag="ps")
                        for k in range(4):
                            nc.tensor.matmul(
                                out=ps[:, 512 * k:512 * (k + 1)],
                                lhsT=lhsT,
                                rhs=src[:, W * h + 512 * k:
                                        W * h + 512 * (k + 1)],
                                start=True, stop=True,
                            )
                        if h == 0:
                            e = ef.tile([P, W], F32, tag="ef")
                        else:
                            e = eh.tile([P, W], BF16, tag="eh")
                        nc.scalar.activation(out=e[:], in_=ps[:],
                                             func=ACTF.Exp, scale=INV_T)
                        # row-sum of this phase
                        nc.vector.tensor_reduce(
                            out=out_s[:, 4 * cc + 2 * si + h:
                                      4 * cc + 2 * si + h + 1],
                            in_=e[:], axis=AX.X, op=ALU.add)
                        if h == 0 and si == 0:
                            # same-track x sum over the window region
                            j = jp.tile([P, REG], F32, tag="junk")
                            nc.vector.tensor_tensor(
                                out=j[:],
                                in0=e[:, off:off + REG],
                                in1=xm_s[:, REG * cc:REG * (cc + 1)],
                                op=ALU.mult)
                            nc.vector.tensor_reduce(
                                out=out_s[:, 16 + cc:17 + cc],
                                in_=j[:], axis=AX.X, op=ALU.add)
                        if h == 0 and si == 1:
                            # own-track y sum (mask holds 10.0 -> 10*own)
                            j = jp.tile([P, REG], F32, tag="junk")
                            nc.vector.tensor_tensor(
                                out=j[:],
                                in0=e[:, off:off + REG],
                                in1=ym_s[:, REG * cc:REG * (cc + 1)],
                                op=ALU.mult)
                            nc.vector.tensor_reduce(
                                out=out_s[:, 20 + cc:21 + cc],
                                in_=j[:], axis=AX.X, op=ALU.add)
                            # min over own views: min(S - 10*mask) = simp-10
                            j2 = jp.tile([P, REG], F32, tag="junk")
                            nc.vector.tensor_tensor(
                                out=j2[:],
                                in0=ps[:, off:off + REG],
                                in1=ym_s[:, REG * cc:REG * (cc + 1)],
                                op=ALU.subtract)
                            nc.vector.tensor_reduce(
                                out=out_s[:, 24 + cc:25 + cc],
                                in_=j2[:], axis=AX.X, op=ALU.min)

            nc.sync.dma_start(out=out_d.ap(), in_=out_s[:])

    nc.compile()
    return nc


def get_nc():
    if "nc" not in _CACHE:
        _CACHE["nc"] = _build()
    return _CACHE["nc"]


def _plan(track_idxs):
    """Sort rows by track; build per-core column orders + masks."""
    t = np.asarray(track_idxs).astype(np.int64)
    perm = np.argsort(t, kind="stable")
    ts = t[perm]                                   # sorted track per row
    tstart = np.searchsorted(ts, np.arange(NT), "left")
    tend = np.searchsorted(ts, np.arange(NT), "right")
    plans = []
    ok = True
    for c in range(CORES):
        tc = ts[c * R:(c + 1) * R]
        seen = set()
        winx = [[] for _ in range(NCH)]
        winy = [[] for _ in range(NCH)]
        for cc in range(NCH):
            for p in range(P):
                tau = int(tc[cc * P + p])
                if tau not in seen:
                    seen.add(tau)
                    winx[cc].extend(range(int(tstart[tau]), int(tend[tau])))
                    winy[cc].extend(range(Q * tau, Q * tau + Q))
            if len(winx[cc]) > WINW or len(winy[cc]) > WINW:
                ok = False
        if not ok:
            break
        cox = np.full(N, -1, np.int64)
        coy = np.full(N, -1, np.int64)
        usedx = np.zeros(N, bool)
        usedy = np.zeros(N, bool)
        for cc in range(NCH):
            cox[WINW * cc:WINW * cc + len(winx[cc])] = winx[cc]
            usedx[winx[cc]] = True
            coy[WINW * cc:WINW * cc + len(winy[cc])] = winy[cc]
            usedy[winy[cc]] = True
        cox[cox < 0] = np.nonzero(~usedx)[0]
        coy[coy < 0] = np.nonzero(~usedy)[0]
        # masks over the per-chunk 512-wide regions
        xm = np.zeros((P, NCH * REG), np.float32)
        ym = np.zeros((P, NCH * REG), np.float32)
        for cc in range(NCH):
            off = OFFS[cc]
            trx = ts[cox[off:off + REG]]           # track of each region col
            try_ = coy[off:off + REG] // Q
            rows = tc[cc * P:(cc + 1) * P]
            mx = trx[None, :] == rows[:, None]
            my = try_[None, :] == rows[:, None]
            # coverage check: every same-track partner must be in-region
            if not (mx.sum(1) == (tend - tstart)[rows]).all():
                ok = False
            if not (my.sum(1) == Q).all():
                ok = False
            xm[:, REG * cc:REG * (cc + 1)] = mx
            ym[:, REG * cc:REG * (cc + 1)] = 10.0 * my
        if not ok:
            break
        plans.append((cox, coy, xm, ym))
    return perm, ts, plans, ok


def prepare_in_maps(x, track_idxs, y):
    x = np.ascontiguousarray(np.asarray(x), dtype=np.float32)
    y = np.ascontiguousarray(np.asarray(y), dtype=np.float32)
    perm, ts, plans, ok = _plan(track_idxs)
    if not ok:
        return None
    xs = x[perm]                                   # sorted rows
    xsT = np.ascontiguousarray(xs.T).astype(ml_dtypes.bfloat16)
    yfT = np.ascontiguousarray(y.reshape(N, D).T).astype(ml_dtypes.bfloat16)
    in_maps = []
    for c in range(CORES):
        cox, coy, xm, ym = plans[c]
        in_maps.append({
            "xT": np.ascontiguousarray(xsT[:, cox]),
            "yT": np.ascontiguousarray(yfT[:, coy]),
            "xsh": np.ascontiguousarray(xsT[:, c * R:(c + 1) * R]),
            "xm": np.ascontiguousarray(xm.astype(ml_dtypes.bfloat16)),
            "ym": np.ascontiguousarray(ym.astype(ml_dtypes.bfloat16)),
        })
    return in_maps


def _exact_fallback(x, track_idxs, y):
    x = np.asarray(x, dtype=np.float64)
    y = np.asarray(y, dtype=np.float64)
    t = np.asarray(track_idxs)
    yf = y.reshape(NT * Q, D)
    ct = np.repeat(np.arange(NT), Q)
    own = t[:, None] == ct[None, :]
    S_xy = x @ yf.T
    sim_p = np.where(own, S_xy, np.inf).min(1)
    den_y = np.where(own, 0.0, np.exp(S_xy / TEMP)).sum(1)
    same = t[:, None] == t[None, :]
    S_xx = x @ x.T
    den_x = np.where(same, 0.0, np.exp(S_xx / TEMP)).sum(1)
    den = den_y + den_x
    num = np.exp(sim_p / TEMP)
    loss = np.log(den[None, :] + num[:, None]).mean() - (sim_p / TEMP).mean()
    return np.float32(loss)


def combine_outputs(outs, inputs=None):
    """outs: per-core [128, 28] partials -> scalar loss via log1p series."""
    den = np.empty(N, np.float64)
    simp = np.empty(N, np.float64)
    ok = True
    for c, o in enumerate(outs):
        o = np.asarray(o, dtype=np.float64)
        if not np.all(np.isfinite(o)):
            ok = False
            break
        for cc in range(NCH):
            g = slice(c * R + cc * P, c * R + (cc + 1) * P)
            tot = o[:, 4 * cc:4 * cc + 4].sum(1)
            den[g] = tot - o[:, 16 + cc] - o[:, 20 + cc] / 10.0
            simp[g] = o[:, 24 + cc] + 10.0
    if ok and np.all(den > 0):
        num = np.exp(simp * INV_T)
        logden = np.log(den).sum()
        K_SER = 4
        terms = []
        for k in range(1, K_SER + 1):
            terms.append((-1.0) ** (k + 1) / k
                         * (num ** k).sum() * (den ** -float(k)).sum())
        pair = N * logden + sum(terms)
        if (abs(terms[-1]) <= 1e-8 * abs(pair) + 1e-12
                and abs(terms[-1]) <= abs(terms[-2]) + 1e-30):
            return np.float32(pair / (N * N) - simp.sum() * INV_T / N)
    if inputs is None:
        raise RuntimeError("device partials unusable and no fallback inputs")
    return _exact_fallback(**inputs)


def kernel(x, track_idxs, y):
    in_maps = prepare_in_maps(x, track_idxs, y)
    if in_maps is None:
        return _exact_fallback(x=x, track_idxs=track_idxs, y=y)
    nc = get_nc()
    res = bass_utils.run_bass_kernel_spmd(nc, in_maps,
                                          core_ids=list(range(CORES)))
    return combine_outputs([r["out"] for r in res.results],
                           inputs={"x": x, "track_idxs": track_idxs, "y": y})


if __name__ == "__main__":
    nc = get_nc()
    print("build + compile OK")


# revision 7
# speedup vs baseline: 1.2914x; 1.2914x over previous
"""Trainium2 Bass kernel for nn_ContrastiveLoss (N=4096, D=128, NT=512, Q=8).

Strategy (8 NeuronCores, data parallel over N, no cross-core collective):
  - Host sorts rows of x by track id (the loss is invariant under a common
    permutation of rows) and, per core, permutes the COLUMN order of both
    similarity matrices so that every data-dependent gather becomes a
    compile-time-static slice: each 128-row chunk's same-track x columns
    and own-track y view columns land in a fixed 512-wide window region.
  - Device work per core is a 16-phase pipeline: [128,2048] matmul (bf16,
    K=D=128) -> exp on ACT -> row totals.  Clean phases accumulate on the
    ACT accumulator (free with the exp); the xx phase containing the
    diagonal e^{1/T} spike is summed via DVE scalar_tensor_tensor with an
    INVERTED same-track mask, so the e^20 diagonal never enters any sum
    (no catastrophic accumulation).  sim_p comes from min(S - 10*ownmask)
    on a static window slice of the xy PSUM tile.
  - The device ships raw per-row partials [128,32]; the host assembles
    den/num in float64 and evaluates the loss with a log1p series
    (validated, exact numpy fallback retained).
"""

import numpy as np
import ml_dtypes

import concourse.bass as bass
import concourse.bacc as bacc
import concourse.tile as tile
import concourse.mybir as mybir
from concourse import bass_utils

P = 128           # partitions / rows per chunk
N = 4096          # total rows of x
D = 128           # feature dim
NT = 512          # number of tracks
Q = 8             # views per track
CORES = 8
R = N // CORES    # rows per core = 512
NCH = R // P      # chunks per core = 4
TEMP = 0.05
INV_T = 1.0 / TEMP
W = 2048          # phase width (4 PSUM banks)
WINW = 256        # per-chunk window width
REG = 512         # mask region width (two windows)
OFFS = (0, 0, 256, 512)   # mask region start per chunk
# spans of the xx-h0 tile outside the mask region (per chunk, <=2)
SPANS = (
    ((512, 2048), None),
    ((512, 2048), None),
    ((0, 256), (768, 2048)),
    ((0, 512), (1024, 2048)),
)
OUTC = 32
F32 = mybir.dt.float32
BF16 = mybir.dt.bfloat16
AX = mybir.AxisListType
ALU = mybir.AluOpType
ACTF = mybir.ActivationFunctionType

_CACHE = {}


def _build():
    nc = bacc.Bacc("TRN2", target_bir_lowering=False, debug=False,
                   num_devices=CORES)

    xT_d = nc.dram_tensor("xT", [D, N], BF16, kind="ExternalInput")
    yT_d = nc.dram_tensor("yT", [D, N], BF16, kind="ExternalInput")
    xsh_d = nc.dram_tensor("xsh", [D, R], BF16, kind="ExternalInput")
    xmi_d = nc.dram_tensor("xmi", [P, NCH * REG], BF16, kind="ExternalInput")
    ym_d = nc.dram_tensor("ym", [P, NCH * REG], BF16, kind="ExternalInput")
    out_d = nc.dram_tensor("out", [P, OUTC], F32, kind="ExternalOutput")

    with tile.TileContext(nc) as tc:
        with (
            tc.tile_pool(name="persist", bufs=1) as pp,
            tc.tile_pool(name="etile", bufs=3) as ep,
            tc.tile_pool(name="junk", bufs=2) as jp,
            tc.tile_pool(name="psum", bufs=2, space="PSUM") as psp,
        ):
            xT_s = pp.tile([D, N], BF16, tag="xT_s")
            yT_s = pp.tile([D, N], BF16, tag="yT_s")
            xsh_s = pp.tile([D, R], BF16, tag="xsh_s")
            xmi_s = pp.tile([P, NCH * REG], BF16, tag="xmi_s")
            ym_s = pp.tile([P, NCH * REG], BF16, tag="ym_s")
            out_s = pp.tile([P, OUTC], F32, tag="out_s")

            nc.vector.memset(out_s[:], 0.0)

            # input loads, in consumption order (single HWDGE queue)
            nc.sync.dma_start(out=xsh_s[:], in_=xsh_d.ap())
            nc.sync.dma_start(out=xT_s[:, 0:W], in_=xT_d.ap()[:, 0:W])
            nc.sync.dma_start(out=xT_s[:, W:N], in_=xT_d.ap()[:, W:N])
            nc.sync.dma_start(out=yT_s[:, 0:W], in_=yT_d.ap()[:, 0:W])
            nc.sync.dma_start(out=xmi_s[:], in_=xmi_d.ap())
            nc.sync.dma_start(out=yT_s[:, W:N], in_=yT_d.ap()[:, W:N])
            nc.sync.dma_start(out=ym_s[:], in_=ym_d.ap())

            for cc in range(NCH):
                lhsT = xsh_s[:, cc * P:(cc + 1) * P]
                off = OFFS[cc]
                reg = slice(REG * cc, REG * (cc + 1))
                for si, src in enumerate((xT_s, yT_s)):
                    for h in range(2):
                        ps = psp.tile([P, W], F32, tag="ps")
                        for k in range(4):
                            nc.tensor.matmul(
                                out=ps[:, 512 * k:512 * (k + 1)],
                                lhsT=lhsT,
                                rhs=src[:, W * h + 512 * k:
                                        W * h + 512 * (k + 1)],
                                start=True, stop=True,
                            )
                        e = ep.tile([P, W], BF16, tag="etile")
                        if si == 0 and h == 0:
                            # dirty phase: no ACT accum; DVE sums below
                            nc.scalar.activation(out=e[:], in_=ps[:],
                                                 func=ACTF.Exp, scale=INV_T)
                            for sp, spi in zip(SPANS[cc], (0, 1)):
                                if sp is None:
                                    continue
                                a, b = sp
                                j = jp.tile([P, W], BF16, tag="junk")
                                nc.vector.scalar_tensor_tensor(
                                    out=j[:, 0:b - a],
                                    in0=e[:, a:b], scalar=0.0,
                                    in1=e[:, a:b],
                                    op0=ALU.max, op1=ALU.max,
                                    accum_out=out_s[:, 16 + 2 * cc + spi:
                                                    17 + 2 * cc + spi])
                            j = jp.tile([P, W], BF16, tag="junk")
                            nc.vector.scalar_tensor_tensor(
                                out=j[:, 0:REG],
                                in0=e[:, off:off + REG], scalar=0.0,
                                in1=xmi_s[:, reg],
                                op0=ALU.max, op1=ALU.mult,
                                accum_out=out_s[:, 24 + cc:25 + cc])
                        else:
                            # clean phases: free row total on ACT accum
                            nc.scalar.activation(
                                out=e[:], in_=ps[:], func=ACTF.Exp,
                                scale=INV_T,
                                accum_out=out_s[:, 4 * cc + 2 * si + h - 1:
                                                4 * cc + 2 * si + h])
                        if si == 1 and h == 0:
                            # own-track y sum (mask holds 10.0 -> 10*own)
                            j = jp.tile([P, W], BF16, tag="junk")
                            nc.vector.scalar_tensor_tensor(
                                out=j[:, 0:REG],
                                in0=e[:, off:off + REG], scalar=0.0,
                                in1=ym_s[:, reg],
                                op0=ALU.max, op1=ALU.mult,
                                accum_out=out_s[:, 28 + cc:29 + cc])
                            # min over own views: min(S - 10*mask) = simp-10
                            j2 = jp.tile([P, REG], F32, tag="junkf")
                            nc.vector.tensor_tensor(
                                out=j2[:],
                                in0=ps[:, off:off + REG],
                                in1=ym_s[:, reg],
                                op=ALU.subtract)
                            nc.vector.tensor_reduce(
                                out=out_s[:, 4 * cc + 3:4 * cc + 4],
                                in_=j2[:], axis=AX.X, op=ALU.min)

            nc.sync.dma_start(out=out_d.ap(), in_=out_s[:])

    nc.compile()
    return nc


def get_nc():
    if "nc" not in _CACHE:
        _CACHE["nc"] = _build()
    return _CACHE["nc"]


def _plan(track_idxs):
    """Sort rows by track; build per-core column orders + masks."""
    t = np.asarray(track_idxs).astype(np.int64)
    perm = np.argsort(t, kind="stable")
    ts = t[perm]                                   # sorted track per row
    tstart = np.searchsorted(ts, np.arange(NT), "left")
    tend = np.searchsorted(ts, np.arange(NT), "right")
    plans = []
    ok = True
    for c in range(CORES):
        tc = ts[c * R:(c + 1) * R]
        seen = set()
        winx = [[] for _ in range(NCH)]
        winy = [[] for _ in range(NCH)]
        for cc in range(NCH):
            for p in range(P):
                tau = int(tc[cc * P + p])
                if tau not in seen:
                    seen.add(tau)
                    winx[cc].extend(range(int(tstart[tau]), int(tend[tau])))
                    winy[cc].extend(range(Q * tau, Q * tau + Q))
            if len(winx[cc]) > WINW or len(winy[cc]) > WINW:
                ok = False
        if not ok:
            break
        cox = np.full(N, -1, np.int64)
        coy = np.full(N, -1, np.int64)
        usedx = np.zeros(N, bool)
        usedy = np.zeros(N, bool)
        for cc in range(NCH):
            cox[WINW * cc:WINW * cc + len(winx[cc])] = winx[cc]
            usedx[winx[cc]] = True
            coy[WINW * cc:WINW * cc + len(winy[cc])] = winy[cc]
            usedy[winy[cc]] = True
        cox[cox < 0] = np.nonzero(~usedx)[0]
        coy[coy < 0] = np.nonzero(~usedy)[0]
        # masks over the per-chunk 512-wide regions
        xmi = np.zeros((P, NCH * REG), np.float32)
        ym = np.zeros((P, NCH * REG), np.float32)
        for cc in range(NCH):
            off = OFFS[cc]
            trx = ts[cox[off:off + REG]]           # track of each region col
            try_ = coy[off:off + REG] // Q
            rows = tc[cc * P:(cc + 1) * P]
            mx = trx[None, :] == rows[:, None]
            my = try_[None, :] == rows[:, None]
            # coverage check: every same-track partner must be in-region
            if not (mx.sum(1) == (tend - tstart)[rows]).all():
                ok = False
            if not (my.sum(1) == Q).all():
                ok = False
            xmi[:, REG * cc:REG * (cc + 1)] = 1.0 - mx
            ym[:, REG * cc:REG * (cc + 1)] = 10.0 * my
        if not ok:
            break
        plans.append((cox, coy, xmi, ym))
    return perm, ts, plans, ok


def prepare_in_maps(x, track_idxs, y):
    x = np.ascontiguousarray(np.asarray(x), dtype=np.float32)
    y = np.ascontiguousarray(np.asarray(y), dtype=np.float32)
    perm, ts, plans, ok = _plan(track_idxs)
    if not ok:
        return None
    xs = x[perm]                                   # sorted rows
    xsT = np.ascontiguousarray(xs.T).astype(ml_dtypes.bfloat16)
    yfT = np.ascontiguousarray(y.reshape(N, D).T).astype(ml_dtypes.bfloat16)
    in_maps = []
    for c in range(CORES):
        cox, coy, xmi, ym = plans[c]
        in_maps.append({
            "xT": np.ascontiguousarray(xsT[:, cox]),
            "yT": np.ascontiguousarray(yfT[:, coy]),
            "xsh": np.ascontiguousarray(xsT[:, c * R:(c + 1) * R]),
            "xmi": np.ascontiguousarray(xmi.astype(ml_dtypes.bfloat16)),
            "ym": np.ascontiguousarray(ym.astype(ml_dtypes.bfloat16)),
        })
    return in_maps


def _exact_fallback(x, track_idxs, y):
    x = np.asarray(x, dtype=np.float64)
    y = np.asarray(y, dtype=np.float64)
    t = np.asarray(track_idxs)
    yf = y.reshape(NT * Q, D)
    ct = np.repeat(np.arange(NT), Q)
    own = t[:, None] == ct[None, :]
    S_xy = x @ yf.T
    sim_p = np.where(own, S_xy, np.inf).min(1)
    den_y = np.where(own, 0.0, np.exp(S_xy / TEMP)).sum(1)
    same = t[:, None] == t[None, :]
    S_xx = x @ x.T
    den_x = np.where(same, 0.0, np.exp(S_xx / TEMP)).sum(1)
    den = den_y + den_x
    num = np.exp(sim_p / TEMP)
    loss = np.log(den[None, :] + num[:, None]).mean() - (sim_p / TEMP).mean()
    return np.float32(loss)


def combine_outputs(outs, inputs=None):
    """outs: per-core [128, 32] partials -> scalar loss via log1p series."""
    den = np.empty(N, np.float64)
    simp = np.empty(N, np.float64)
    ok = True
    for c, o in enumerate(outs):
        o = np.asarray(o, dtype=np.float64)
        if not np.all(np.isfinite(o)):
            ok = False
            break
        for cc in range(NCH):
            g = slice(c * R + cc * P, c * R + (cc + 1) * P)
            den[g] = (o[:, 4 * cc] + o[:, 4 * cc + 1] + o[:, 4 * cc + 2]
                      + o[:, 16 + 2 * cc] + o[:, 17 + 2 * cc]
                      + o[:, 24 + cc] - o[:, 28 + cc] / 10.0)
            simp[g] = o[:, 4 * cc + 3] + 10.0
    if ok and np.all(den > 0):
        num = np.exp(simp * INV_T)
        logden = np.log(den).sum()
        K_SER = 4
        terms = []
        for k in range(1, K_SER + 1):
            terms.append((-1.0) ** (k + 1) / k
                         * (num ** k).sum() * (den ** -float(k)).sum())
        pair = N * logden + sum(terms)
        if (abs(terms[-1]) <= 1e-8 * abs(pair) + 1e-12
                and abs(terms[-1]) <= abs(terms[-2]) + 1e-30):
            return np.float32(pair / (N * N) - simp.sum() * INV_T / N)
    if inputs is None:
        raise RuntimeError("device partials unusable and no fallback inputs")
    return _exact_fallback(**inputs)


def kernel(x, track_idxs, y):
    in_maps = prepare_in_maps(x, track_idxs, y)
    if in_maps is None:
        return _exact_fallback(x=x, track_idxs=track_idxs, y=y)
    nc = get_nc()
    res = bass_utils.run_bass_kernel_spmd(nc, in_maps,
                                          core_ids=list(range(CORES)))
    return combine_outputs([r["out"] for r in res.results],
                           inputs={"x": x, "track_idxs": track_idxs, "y": y})


if __name__ == "__main__":
    nc = get_nc()
    print("build + compile OK")
